# revision 35
# baseline (speedup 1.0000x reference)
"""Trainium2 Bass kernel for nn_CentralizedCritic (pooling critic net).

Data-parallel over 8 NeuronCores: each core handles B_c=2048 batch rows.

Per-core math (matches the jax reference):
  robot_emb = setenc(robot[b], rw*)  -> [B,32]   (mean+max pool over 64)
  track_emb = setenc(track[b], tw*)  -> [B,32]   (mean+max pool over 128)
  c = [tier0, robot_emb, track_emb]  -> [B,108]
  y = mlp(c)                         -> [B]

On-chip mapping:
  - Activations kept transposed [feat, rows]; 2 batch-halves packed on the
    partition dim via block-diag weights (K=2*d_in, M=2*d_hidden=128).
  - x^T pre-packed on host into 4 row-groups at partition offsets {0,32,64,96}
    so L1 matmuls row-tile the PE array. bf16 throughout (rel err ~7e-3).
  - Both branches flattened into one desc list and software-pipelined 3 deep
    (PE per iter: L1(i), L2(i-1), L3(i-2)) so the PE never waits on a
    same-iteration evacuation.
  - relu+bias fused into the PSUM->SBUF evacuation; evacs column-split
    between ACT and DVE (SPLITS) so ACT-busy ~= DVE-busy (reduces are
    DVE-only: ACT and DVE are the only PSUM-capable engines, which sets
    the ~3.6us/tile steady-state floor).
  - mean-pool: DVE reduce_sum from PSUM; max-pool: DVE reduce_max from PSUM.
  - e-bias (rb3/tb3) folded into the head-L1 bias on host.
  - branch combine+scatter issued as soon as its pair-columns are final
    (track fully + robot cols [0,192) hidden under the steady state); head
    runs layer-major with ACT/DVE-alternating evacs so its 4 col-tiles
    pipeline.
"""

import sys

sys.path.insert(0, "/opt/trn_rl_repo")

import numpy as np
import ml_dtypes

import concourse.bass as bass  # noqa: F401  (bass must import before tile)
import concourse.mybir as mybir
import concourse.tile as tile
from concourse import bacc
from concourse.bass_utils import run_bass_kernel_spmd

F32R = mybir.dt.float32r
F32 = mybir.dt.float32
BF16 = mybir.dt.bfloat16
AF = mybir.ActivationFunctionType
ALU = mybir.AluOpType
AX = mybir.AxisListType

N_CORES = 8
B = 16384
B_C = B // N_CORES          # 2048 batch rows per core
HALF = B_C // 2             # 1024 (2-row packing pairs b and b+HALF)
NR, DR = 64, 6              # robot set size / feature dim
NT, DT = 128, 7             # track set size / feature dim
CT = B_C * NT // 2          # 131072 packed track cols per core
CR = B_C * NR // 2          # 65536 packed robot cols per core
QT = CT // 4                # 32768 cols per track row-group
QR = CR // 4                # 16384 cols per robot row-group
CHUNK = int(__import__("os").environ.get("CHUNK", "2048"))  # dma chunk cols
NTILE = 512                 # matmul free dim
# DVE column-split per evac slot [e1h0, e1h1, e2h0, e2h1]: how many of the
# NTILE columns of that evac go to DVE (rest on ACT). 0=all ACT, 512=all DVE.
SPLITS = tuple(int(x) for x in
               __import__("os").environ.get("SPLITS", "0,0,32,320").split(","))
LOOKAHEAD = int(__import__("os").environ.get("LOOKAHEAD", "5"))

# const-block column layout in "wts" [128, 840] (f32r)
W1T_C, W1R_C, W2T_C, W2R_C = 0, 128, 256, 384
MW1_C, MW2_C, MW3_C, MW4_C = 512, 640, 768, 832
WTS_W = 840
# "bs" [128, 8] (f32) bias columns
BS_TB1, BS_RB1, BS_TB2, BS_RB2, BS_MB1, BS_MB2, BS_MB3, BS_MB4 = range(8)

_CACHE = {}

import os
PROBE_NO_REDUCE = os.environ.get("PROBE_NO_REDUCE") == "1"
POOL_EVAC = os.environ.get("POOL_EVAC", "0") == "1"
PROBE_NO_EVAC = os.environ.get("PROBE_NO_EVAC") == "1"
PROBE_NO_MM3 = os.environ.get("PROBE_NO_MM3") == "1"
HBUF_BUFS = int(os.environ.get("HBUF_BUFS", "3"))
XC_BUFS = int(os.environ.get("XC_BUFS", "4"))
PS_BUFS = int(os.environ.get("PS_BUFS", "3"))
PS3_BUFS = int(os.environ.get("PS3_BUFS", "1"))  # 0 = share "ps" pool
DVE_EVAC_MOD = int(os.environ.get("DVE_EVAC_MOD", "3"))
MMDT_NAME = os.environ.get("MMDT", "bf16")
SPLIT_L3 = os.environ.get("SPLIT_L3", "0") == "1"
EVAC_ASSIGN = os.environ.get("EVAC_ASSIGN", "rr")  # rr | e1
MMDT = mybir.dt.bfloat16 if MMDT_NAME == "bf16" else mybir.dt.float32r
MMDT_NP = ml_dtypes.bfloat16 if MMDT_NAME == "bf16" else np.float32


def _build_bass():
    nc = bacc.Bacc("TRN2", target_bir_lowering=False, debug=False,
                   num_devices=N_CORES)
    xt_d = nc.dram_tensor("xt", [128, QT], MMDT, kind="ExternalInput")
    xr_d = nc.dram_tensor("xr", [128, QR], MMDT, kind="ExternalInput")
    t0_d = nc.dram_tensor("t0", [44, B_C], MMDT, kind="ExternalInput")
    wts_d = nc.dram_tensor("wts", [128, WTS_W], MMDT, kind="ExternalInput")
    wbf_d = nc.dram_tensor("wbf", [128, 128], BF16, kind="ExternalInput")
    bs_d = nc.dram_tensor("bs", [128, 8], F32, kind="ExternalInput")
    y_d = nc.dram_tensor("y", [1, B_C], F32, kind="ExternalOutput")

    with tile.TileContext(nc) as tc:
        with (
            tc.tile_pool(name="consts", bufs=1) as consts,
            tc.tile_pool(name="xchunks", bufs=XC_BUFS) as xchunks,
            tc.tile_pool(name="hbuf", bufs=HBUF_BUFS) as hbuf,
            tc.tile_pool(name="acc", bufs=1) as acc,
            tc.tile_pool(name="head", bufs=2) as head,
            tc.tile_pool(name="ps", bufs=PS_BUFS, space="PSUM") as ps,
            tc.tile_pool(name="ps3p", bufs=max(PS3_BUFS, 1),
                         space="PSUM") as ps3p,
        ):
            wts = consts.tile([128, WTS_W], MMDT)
            wbf = consts.tile([128, 128], BF16)
            bs = consts.tile([128, 8], F32)
            nc.sync.dma_start(out=wts[:, 0:256], in_=wts_d[:, 0:256])
            nc.sync.dma_start(out=bs[:], in_=bs_d[:])

            cT = acc.tile([108, B_C], MMDT)

            # pooled accumulators: col = 256*J + 32*ch + 4*t + om (track)
            #                      col = 256*J + 64*ch + 8*t + om (robot)
            esum_t = acc.tile([128, 512], F32, tag="esum_t")
            emax_t = acc.tile([128, 512], F32, tag="emax_t")
            esum_r = acc.tile([128, 512], F32, tag="esum_r")
            emax_r = acc.tile([128, 512], F32, tag="emax_r")

            # ---- unified descriptor list over both branches ----
            # each desc = one NTILE-wide tile step of a branch
            descs = []
            chunks = []  # (x_d, col0, first_desc_idx)
            for (x_d, qcols, k2, w1_c, w2_c, w3_c, bs1, bs2, nseg,
                 esum, emax) in (
                    (xt_d, QT, 2 * DT, W1T_C, W2T_C, 0, BS_TB1, BS_TB2, NT,
                     esum_t, emax_t),
                    (xr_d, QR, 2 * DR, W1R_C, W2R_C, 64, BS_RB1, BS_RB2, NR,
                     esum_r, emax_r)):
                nchunks = qcols // CHUNK
                tpc = CHUNK // NTILE
                tg = 0
                for ch in range(nchunks):
                    chunks.append((x_d, ch * CHUNK, len(descs)))
                    for t in range(tpc):
                        descs.append(dict(
                            chunk=len(chunks) - 1, cs=slice(t * NTILE,
                                                            (t + 1) * NTILE),
                            k2=k2, w1_c=w1_c, w2_c=w2_c, w3_c=w3_c,
                            bs1=bs1, bs2=bs2, nseg=nseg, esum=esum,
                            emax=emax, nb=NTILE // nseg,
                            base=(NTILE // nseg) * tg))
                        tg += 1

            chunk_tiles = {}
            issued = [0]  # chunks issued so far

            def issue_chunk(ci):
                x_d, c0, _ = chunks[ci]
                xc = xchunks.tile([128, CHUNK], MMDT, tag="xc")
                nc.sync.dma_start(out=xc[:], in_=x_d[:, c0:c0 + CHUNK])
                chunk_tiles[ci] = xc

            def evac(pshalf, htile, half, bias_col, dve_cols):
                """relu+bias PSUM->SBUF; column-split ACT/DVE."""
                ac = NTILE - dve_cols
                if dve_cols > 0:
                    eng = nc.gpsimd if POOL_EVAC else nc.vector
                    eng.tensor_scalar(
                        out=htile[:, 2 * half:2 * half + 2, ac:NTILE],
                        in0=pshalf[:, :, ac:NTILE],
                        scalar1=bs[:, bias_col:bias_col + 1],
                        scalar2=0.0, op0=ALU.add, op1=ALU.max)
                if ac > 0:
                    nc.scalar.activation(
                        out=htile[:, 2 * half:2 * half + 2, 0:ac],
                        in_=pshalf[:, :, 0:ac], func=AF.Relu,
                        bias=bs[:, bias_col:bias_col + 1], scale=1.0)

            def stage1(d):
                """L1 matmuls + evac1 -> h1."""
                ci = d["chunk"]
                while issued[0] <= min(ci + 1, len(chunks) - 1):
                    issue_chunk(issued[0])
                    issued[0] += 1
                xc = chunk_tiles[ci]
                h1 = hbuf.tile([128, 4, NTILE], MMDT, tag="h1")
                d["h1"] = h1
                pstiles = []
                for half in range(2):
                    p = ps.tile([128, 2, NTILE], F32, tag="ps")
                    pstiles.append(p)
                    for j in range(2):
                        q = 2 * half + j
                        nc.tensor.matmul(
                            p[:, j, :],
                            wts[32 * q:32 * q + d["k2"],
                                d["w1_c"]:d["w1_c"] + 128],
                            xc[32 * q:32 * q + d["k2"], d["cs"]],
                            start=True, stop=True,
                            tile_position=(32 * q, 0))
                for half in range(2):
                    evac(pstiles[half], h1, half, d["bs1"], SPLITS[half])

            def stage2(d):
                """L2 matmuls + evac2 -> h2."""
                h1 = d.pop("h1")
                h2 = hbuf.tile([128, 4, NTILE], BF16, tag="h2")
                d["h2"] = h2
                pstiles = []
                for half in range(2):
                    p = ps.tile([128, 2, NTILE], F32, tag="ps")
                    pstiles.append(p)
                    for j in range(2):
                        q = 2 * half + j
                        nc.tensor.matmul(
                            p[:, j, :],
                            wts[:, d["w2_c"]:d["w2_c"] + 128],
                            h1[:, q, :], start=True, stop=True)
                for half in range(2):
                    evac(pstiles[half], h2, half, d["bs2"], SPLITS[2 + half])

            def stage3(d):
                """L3 (col-paired) + pooling reduces."""
                h2 = d.pop("h2")
                nb, nseg = d["nb"], d["nseg"]
                pool3 = ps3p if PS3_BUFS > 0 else ps
                ps3 = pool3.tile([128, 2 * nb * nseg], F32,
                                 tag="ps3" if PS3_BUFS else "ps")
                for q in range(4):
                    J, blk = q // 2, q % 2
                    nc.tensor.matmul(
                        ps3[64 * blk:64 * blk + 64,
                            J * NTILE:(J + 1) * NTILE],
                        wbf[:, d["w3_c"]:d["w3_c"] + 64],
                        h2[:, q, :], start=True, stop=True,
                        tile_position=(0, 64 * blk))
                p3r = ps3.rearrange("p (a b c) -> p (a b) c", a=2, b=nb)
                base = d["base"]
                sview = d["esum"].rearrange("p (J r) -> p J r", J=2)[
                    :, :, base:base + nb]
                mview = d["emax"].rearrange("p (J r) -> p J r", J=2)[
                    :, :, base:base + nb]
                if not PROBE_NO_REDUCE:
                    nc.vector.reduce_sum(out=sview, in_=p3r[:], axis=AX.X)
                    nc.vector.reduce_max(out=mview, in_=p3r[:], axis=AX.X)

            emb_t = acc.tile([128, 512], MMDT, tag="emb_t")
            emb_r = acc.tile([128, 512], MMDT, tag="emb_r")

            def combine_and_scatter(esum, emax, emb, nseg, row0, p0, p1,
                                    parallel_q=False):
                """emb = esum/(2*nseg) + 0.5*emax for pair-cols [p0, p1) of
                each J half, then scatter into cT. Split so most of it can
                issue before a branch's last reduce."""
                npair = p1 - p0
                eview = lambda t: t.rearrange("p (J x) -> p J x", J=2)[
                    :, :, p0:p1]
                tmp = hbuf.tile([128, 512], F32, tag="tmp")
                tv = tmp.rearrange("p (J x) -> p J x", J=2)[:, :, p0:p1]
                nc.vector.tensor_scalar(out=tv, in0=eview(esum),
                                        scalar1=1.0 / (2.0 * nseg),
                                        scalar2=None, op0=ALU.mult)
                nc.vector.scalar_tensor_tensor(
                    out=eview(emb), in0=eview(emax), scalar=0.5, in1=tv,
                    op0=ALU.mult, op1=ALU.add)
                # scatter, J-dim merged: 4 DMAs per call
                embJ = emb.rearrange("p (J x) -> p J x", J=2)[:, :, p0:p1]
                cTv = cT[row0:row0 + 32, :].rearrange(
                    "p (h J blk x) -> p h J blk x", h=2, J=2, blk=2)
                engs = ([nc.sync, nc.scalar, nc.gpsimd, nc.sync]
                        if parallel_q else [nc.sync] * 4)
                for blk in range(2):
                    for h in range(2):
                        engs[2 * blk + h].dma_start(
                            out=cTv[:, h, :, blk, p0:p1],
                            in_=embJ[64 * blk + 32 * h:
                                     64 * blk + 32 * h + 32, :, :])

            n = len(descs)
            ntrack = QT // NTILE  # first ntrack descs are the track branch
            # prime the pipeline: x chunks first, then non-critical consts
            issue_chunk(0)
            issue_chunk(1)
            issued[0] = 2
            nc.sync.dma_start(out=wts[:, 256:WTS_W], in_=wts_d[:, 256:WTS_W])
            nc.sync.dma_start(out=wbf[:], in_=wbf_d[:])
            nc.sync.dma_start(out=cT[0:44, :], in_=t0_d[:])
            for i in range(n + 2):
                if i < n:
                    stage1(descs[i])
                if 1 <= i <= n:
                    stage2(descs[i - 1])
                if 2 <= i <= n + 1:
                    stage3(descs[i - 2])
                if i - 2 == ntrack - 1:
                    combine_and_scatter(esum_t, emax_t, emb_t, NT, 76,
                                        0, 256)
                if i - 2 == ntrack + 23:
                    # robot pair-cols [0,192) are final after robot desc 23
                    combine_and_scatter(esum_r, emax_r, emb_r, NR, 44,
                                        0, 192)
            combine_and_scatter(esum_r, emax_r, emb_r, NR, 44, 192, 256,
                                parallel_q=True)

            # keep the PE p-state warm across the drain->head gap with
            # dummy matmuls on resident constants (results never read)
            NWARM = int(os.environ.get("NWARM", "0"))
            if NWARM:
                warm = ps.tile([128, 2, NTILE], F32, tag="ps")
                for w in range(NWARM):
                    nc.tensor.matmul(warm[:, w % 2, :], wts[:, 0:128],
                                     wts[:, 256:768], start=True, stop=True)

            # ---- head MLP 108 -> 128 -> 128 -> 64 -> 1, layer-major ----
            # evacs alternate ACT/DVE so the 4 col-tiles pipeline.
            y_sb = acc.tile([1, B_C], F32, tag="y")
            NHT = B_C // NTILE  # 4 col-tiles

            def head_evac(pst, dsts, s, bias_col, prange=128, func=AF.Relu):
                if s % 2 == 0:
                    nc.scalar.activation(
                        out=dsts, in_=pst[:], func=func,
                        bias=bs[0:prange, bias_col:bias_col + 1], scale=1.0)
                elif func == AF.Relu:
                    nc.vector.tensor_scalar(
                        out=dsts, in0=pst[:],
                        scalar1=bs[0:prange, bias_col:bias_col + 1],
                        scalar2=0.0, op0=ALU.add, op1=ALU.max)
                else:
                    nc.vector.tensor_scalar(
                        out=dsts, in0=pst[:],
                        scalar1=bs[0:prange, bias_col:bias_col + 1],
                        scalar2=None, op0=ALU.add)

            hh1 = head.tile([128, B_C], MMDT, tag="hh1")
            hh2 = head.tile([128, B_C], MMDT, tag="hh2")
            hh3 = head.tile([64, B_C], MMDT, tag="hh3")
            layers = (
                (lambda s: wts[0:108, MW1_C:MW1_C + 128],
                 lambda s: cT[:, s * NTILE:(s + 1) * NTILE],
                 hh1, BS_MB1, 128),
                (lambda s: wts[:, MW2_C:MW2_C + 128],
                 lambda s: hh1[:, s * NTILE:(s + 1) * NTILE],
                 hh2, BS_MB2, 128),
                (lambda s: wts[:, MW3_C:MW3_C + 64],
                 lambda s: hh2[:, s * NTILE:(s + 1) * NTILE],
                 hh3, BS_MB3, 64),
            )
            for wfn, infn, dst, bcol, prange in layers:
                psl = []
                for s in range(NHT):
                    p = ps.tile([prange, NTILE], F32, tag="ps")
                    psl.append(p)
                    nc.tensor.matmul(p[:], wfn(s), infn(s),
                                     start=True, stop=True)
                for s in range(NHT):
                    head_evac(psl[s],
                              dst[0:prange, s * NTILE:(s + 1) * NTILE],
                              s, bcol, prange)
            psl = []
            for s in range(NHT):
                p = ps.tile([1, NTILE], F32, tag="ps")
                psl.append(p)
                nc.tensor.matmul(p[:], wts[0:64, MW4_C:MW4_C + 1],
                                 hh3[:, s * NTILE:(s + 1) * NTILE],
                                 start=True, stop=True)
            for s in range(NHT):
                head_evac(psl[s], y_sb[:, s * NTILE:(s + 1) * NTILE],
                          s, BS_MB4, 1, AF.Identity)
            nc.sync.dma_start(out=y_d[:], in_=y_sb[:])

    nc.compile()
    return nc


def _pack_x(x, d, qcols):
    """x [rows, d] (rows = B_c*nseg, b-major) -> [128, qcols] with 4
    row-groups at partition offsets {0,32,64,96}; 2-row packing pairs
    row r with row r + rows/2."""
    rows = x.shape[0]
    half = rows // 2
    packed = np.concatenate([x[:half].T, x[half:].T], axis=0)  # [2d, half]
    out = np.zeros((128, qcols), dtype=MMDT_NP)
    for q in range(4):
        out[32 * q:32 * q + 2 * d] = packed[:, q * qcols:(q + 1) * qcols]
    return np.ascontiguousarray(out)


def _blockdiag2(w):
    """w [d, m] -> [2d, 2m] block-diagonal."""
    d, m = w.shape
    out = np.zeros((2 * d, 2 * m), dtype=np.float32)
    out[:d, :m] = w
    out[d:, m:] = w
    return out


def _build_consts(i):
    np32 = lambda a: np.asarray(a, dtype=np.float32)
    wts = np.zeros((128, WTS_W), dtype=np.float32)
    # L1 lhsT blocks replicated at the 4 row-group offsets
    bd1t = _blockdiag2(np32(i["tw1"]))   # [14, 128]
    bd1r = _blockdiag2(np32(i["rw1"]))   # [12, 128]
    for q in range(4):
        wts[32 * q:32 * q + 14, W1T_C:W1T_C + 128] = bd1t
        wts[32 * q:32 * q + 12, W1R_C:W1R_C + 128] = bd1r
    wts[:, W2T_C:W2T_C + 128] = _blockdiag2(np32(i["tw2"]))
    wts[:, W2R_C:W2R_C + 128] = _blockdiag2(np32(i["rw2"]))
    wts[0:108, MW1_C:MW1_C + 128] = np32(i["mw1"])
    wts[:, MW2_C:MW2_C + 128] = np32(i["mw2"])
    wts[:, MW3_C:MW3_C + 64] = np32(i["mw3"])
    wts[0:64, MW4_C:MW4_C + 1] = np32(i["mw4"])
    wts = wts.astype(MMDT_NP)

    wbf = np.zeros((128, 128), dtype=np.float32)
    wbf[:, 0:64] = _blockdiag2(np32(i["tw3"]))
    wbf[:, 64:128] = _blockdiag2(np32(i["rw3"]))
    wbf = wbf.astype(ml_dtypes.bfloat16)

    bs = np.zeros((128, 8), dtype=np.float32)
    bs[:, BS_TB1] = np.concatenate([np32(i["tb1"]), np32(i["tb1"])])
    bs[:, BS_RB1] = np.concatenate([np32(i["rb1"]), np32(i["rb1"])])
    bs[:, BS_TB2] = np.concatenate([np32(i["tb2"]), np32(i["tb2"])])
    bs[:, BS_RB2] = np.concatenate([np32(i["rb2"]), np32(i["rb2"])])
    # fold pooled e-bias into head L1 bias: c@mw1 picks up b3@mw1 rows
    mb1p = (np32(i["mb1"])
            + np32(i["rb3"]) @ np32(i["mw1"])[44:76]
            + np32(i["tb3"]) @ np32(i["mw1"])[76:108])
    bs[:, BS_MB1] = mb1p
    bs[:, BS_MB2] = np32(i["mb2"])
    bs[0:64, BS_MB3] = np32(i["mb3"])
    bs[0:1, BS_MB4] = np32(i["mb4"])
    return wts, wbf, bs


def kernel(**inputs) -> np.ndarray:
    if "nc" not in _CACHE:
        _CACHE["nc"] = _build_bass()
    nc = _CACHE["nc"]

    wts, wbf, bs = _build_consts(inputs)
    t0 = np.asarray(inputs["tier0_features"], dtype=np.float32)
    rb = np.asarray(inputs["robot_features"], dtype=np.float32)
    tk = np.asarray(inputs["track_features"], dtype=np.float32)

    in_maps = []
    for c in range(N_CORES):
        s = slice(c * B_C, (c + 1) * B_C)
        in_maps.append({
            "xt": _pack_x(tk[s].reshape(B_C * NT, DT), DT, QT),
            "xr": _pack_x(rb[s].reshape(B_C * NR, DR), DR, QR),
            "t0": np.ascontiguousarray(t0[s].T).astype(MMDT_NP),
            "wts": wts, "wbf": wbf, "bs": bs,
        })

    res = run_bass_kernel_spmd(nc, in_maps, core_ids=list(range(N_CORES)))
    out = np.concatenate([r["y"][0] for r in res.results])
    return out.astype(np.float32)


if __name__ == "__main__":
    rng = np.random.default_rng(0)
    fake = {
        "tier0_features": rng.standard_normal((B, 44), dtype=np.float32),
        "robot_features": rng.standard_normal((B, NR, DR), dtype=np.float32),
        "track_features": rng.standard_normal((B, NT, DT), dtype=np.float32),
    }
    for n, sh in (("rw1", (6, 64)), ("rw2", (64, 64)), ("rw3", (64, 32)),
                  ("tw1", (7, 64)), ("tw2", (64, 64)), ("tw3", (64, 32)),
                  ("mw1", (108, 128)), ("mw2", (128, 128)),
                  ("mw3", (128, 64)), ("mw4", (64, 1))):
        fake[n] = rng.standard_normal(sh, dtype=np.float32) * 0.2
    for n, sh in (("rb1", 64), ("rb2", 64), ("rb3", 32),
                  ("tb1", 64), ("tb2", 64), ("tb3", 32),
                  ("mb1", 128), ("mb2", 128), ("mb3", 64), ("mb4", 1)):
        fake[n] = rng.standard_normal((sh,), dtype=np.float32) * 0.1
    y = kernel(**fake)
    print("kernel out:", y.shape, y[:4])



# revision 36
# speedup vs baseline: 1.0018x; 1.0018x over previous
"""Trainium2 Bass kernel for nn_CentralizedCritic (pooling critic net).

Data-parallel over 8 NeuronCores: each core handles B_c=2048 batch rows.

Per-core math (matches the jax reference):
  robot_emb = setenc(robot[b], rw*)  -> [B,32]   (mean+max pool over 64)
  track_emb = setenc(track[b], tw*)  -> [B,32]   (mean+max pool over 128)
  c = [tier0, robot_emb, track_emb]  -> [B,108]
  y = mlp(c)                         -> [B]

On-chip mapping:
  - Activations kept transposed [feat, rows]; 2 batch-halves packed on the
    partition dim via block-diag weights (K=2*d_in, M=2*d_hidden=128).
  - x^T pre-packed on host into 4 row-groups at partition offsets {0,32,64,96}
    so L1 matmuls row-tile the PE array. bf16 throughout (rel err ~7e-3).
  - Both branches flattened into one desc list and software-pipelined 3 deep
    (PE per iter: L1(i), L2(i-1), L3(i-2)) so the PE never waits on a
    same-iteration evacuation.
  - relu+bias fused into the PSUM->SBUF evacuation; evacs column-split
    between ACT and DVE (SPLITS) so ACT-busy ~= DVE-busy (reduces are
    DVE-only: ACT and DVE are the only PSUM-capable engines, which sets
    the ~3.6us/tile steady-state floor).
  - mean-pool: DVE reduce_sum from PSUM; max-pool: DVE reduce_max from PSUM.
  - e-bias (rb3/tb3) folded into the head-L1 bias on host.
  - branch combine+scatter issued as soon as its pair-columns are final
    (track fully + robot cols [0,192) hidden under the steady state); head
    runs layer-major with ACT/DVE-alternating evacs so its 4 col-tiles
    pipeline.
"""

import sys

sys.path.insert(0, "/opt/trn_rl_repo")

import numpy as np
import ml_dtypes

import concourse.bass as bass  # noqa: F401  (bass must import before tile)
import concourse.mybir as mybir
import concourse.tile as tile
from concourse import bacc
from concourse.bass_utils import run_bass_kernel_spmd

F32R = mybir.dt.float32r
F32 = mybir.dt.float32
BF16 = mybir.dt.bfloat16
AF = mybir.ActivationFunctionType
ALU = mybir.AluOpType
AX = mybir.AxisListType

N_CORES = 8
B = 16384
B_C = B // N_CORES          # 2048 batch rows per core
HALF = B_C // 2             # 1024 (2-row packing pairs b and b+HALF)
NR, DR = 64, 6              # robot set size / feature dim
NT, DT = 128, 7             # track set size / feature dim
CT = B_C * NT // 2          # 131072 packed track cols per core
CR = B_C * NR // 2          # 65536 packed robot cols per core
QT = CT // 4                # 32768 cols per track row-group
QR = CR // 4                # 16384 cols per robot row-group
CHUNK = int(__import__("os").environ.get("CHUNK", "2048"))  # dma chunk cols
NTILE = 512                 # matmul free dim
# DVE column-split per evac slot [e1h0, e1h1, e2h0, e2h1]: how many of the
# NTILE columns of that evac go to DVE (rest on ACT). 0=all ACT, 512=all DVE.
SPLITS = tuple(int(x) for x in
               __import__("os").environ.get("SPLITS", "0,0,32,320").split(","))
LOOKAHEAD = int(__import__("os").environ.get("LOOKAHEAD", "5"))

# const-block column layout in "wts" [128, 840] (f32r)
W1T_C, W1R_C, W2T_C, W2R_C = 0, 128, 256, 384
MW1_C, MW2_C, MW3_C, MW4_C = 512, 640, 768, 832
WTS_W = 840
# "bs" [128, 8] (f32) bias columns
BS_TB1, BS_RB1, BS_TB2, BS_RB2, BS_MB1, BS_MB2, BS_MB3, BS_MB4 = range(8)

_CACHE = {}

import os
PROBE_NO_REDUCE = os.environ.get("PROBE_NO_REDUCE") == "1"
POOL_EVAC = os.environ.get("POOL_EVAC", "0") == "1"
PROBE_NO_EVAC = os.environ.get("PROBE_NO_EVAC") == "1"
PROBE_NO_MM3 = os.environ.get("PROBE_NO_MM3") == "1"
HBUF_BUFS = int(os.environ.get("HBUF_BUFS", "3"))
XC_BUFS = int(os.environ.get("XC_BUFS", "4"))
PS_BUFS = int(os.environ.get("PS_BUFS", "3"))
PS3_BUFS = int(os.environ.get("PS3_BUFS", "1"))  # 0 = share "ps" pool
DVE_EVAC_MOD = int(os.environ.get("DVE_EVAC_MOD", "3"))
MMDT_NAME = os.environ.get("MMDT", "bf16")
SPLIT_L3 = os.environ.get("SPLIT_L3", "0") == "1"
EVAC_ASSIGN = os.environ.get("EVAC_ASSIGN", "rr")  # rr | e1
MMDT = mybir.dt.bfloat16 if MMDT_NAME == "bf16" else mybir.dt.float32r
MMDT_NP = ml_dtypes.bfloat16 if MMDT_NAME == "bf16" else np.float32


def _build_bass():
    nc = bacc.Bacc("TRN2", target_bir_lowering=False, debug=False,
                   num_devices=N_CORES)
    xt_d = nc.dram_tensor("xt", [128, QT], MMDT, kind="ExternalInput")
    xr_d = nc.dram_tensor("xr", [128, QR], MMDT, kind="ExternalInput")
    t0_d = nc.dram_tensor("t0", [44, B_C], MMDT, kind="ExternalInput")
    wts_d = nc.dram_tensor("wts", [128, WTS_W], MMDT, kind="ExternalInput")
    wbf_d = nc.dram_tensor("wbf", [128, 128], BF16, kind="ExternalInput")
    bs_d = nc.dram_tensor("bs", [128, 8], F32, kind="ExternalInput")
    y_d = nc.dram_tensor("y", [1, B_C], F32, kind="ExternalOutput")

    with tile.TileContext(nc) as tc:
        with (
            tc.tile_pool(name="consts", bufs=1) as consts,
            tc.tile_pool(name="xchunks", bufs=XC_BUFS) as xchunks,
            tc.tile_pool(name="hbuf", bufs=HBUF_BUFS) as hbuf,
            tc.tile_pool(name="acc", bufs=1) as acc,
            tc.tile_pool(name="head", bufs=2) as head,
            tc.tile_pool(name="ps", bufs=PS_BUFS, space="PSUM") as ps,
            tc.tile_pool(name="ps3p", bufs=max(PS3_BUFS, 1),
                         space="PSUM") as ps3p,
        ):
            wts = consts.tile([128, WTS_W], MMDT)
            wbf = consts.tile([128, 128], BF16)
            bs = consts.tile([128, 8], F32)
            nc.sync.dma_start(out=wts[:, 0:256], in_=wts_d[:, 0:256])
            nc.sync.dma_start(out=bs[:], in_=bs_d[:])

            cT = acc.tile([108, B_C], MMDT)

            # pooled accumulators: col = 256*J + 32*ch + 4*t + om (track)
            #                      col = 256*J + 64*ch + 8*t + om (robot)
            esum_t = acc.tile([128, 512], F32, tag="esum_t")
            emax_t = acc.tile([128, 512], F32, tag="emax_t")
            esum_r = acc.tile([128, 512], F32, tag="esum_r")
            emax_r = acc.tile([128, 512], F32, tag="emax_r")

            # ---- unified descriptor list over both branches ----
            # each desc = one NTILE-wide tile step of a branch
            descs = []
            chunks = []  # (x_d, col0, first_desc_idx)
            for (x_d, qcols, k2, w1_c, w2_c, w3_c, bs1, bs2, nseg,
                 esum, emax) in (
                    (xt_d, QT, 2 * DT, W1T_C, W2T_C, 0, BS_TB1, BS_TB2, NT,
                     esum_t, emax_t),
                    (xr_d, QR, 2 * DR, W1R_C, W2R_C, 64, BS_RB1, BS_RB2, NR,
                     esum_r, emax_r)):
                nchunks = qcols // CHUNK
                tpc = CHUNK // NTILE
                tg = 0
                for ch in range(nchunks):
                    chunks.append((x_d, ch * CHUNK, len(descs)))
                    for t in range(tpc):
                        descs.append(dict(
                            chunk=len(chunks) - 1, cs=slice(t * NTILE,
                                                            (t + 1) * NTILE),
                            k2=k2, w1_c=w1_c, w2_c=w2_c, w3_c=w3_c,
                            bs1=bs1, bs2=bs2, nseg=nseg, esum=esum,
                            emax=emax, nb=NTILE // nseg,
                            base=(NTILE // nseg) * tg))
                        tg += 1

            chunk_tiles = {}
            issued = [0]  # chunks issued so far

            def issue_chunk(ci):
                x_d, c0, _ = chunks[ci]
                xc = xchunks.tile([128, CHUNK], MMDT, tag="xc")
                if ci == 0:
                    # split the first chunk so desc 0 unblocks after the
                    # first half lands (range-level dep tracking)
                    h = CHUNK // 2
                    nc.sync.dma_start(out=xc[:, 0:h], in_=x_d[:, c0:c0 + h])
                    nc.sync.dma_start(out=xc[:, h:CHUNK],
                                      in_=x_d[:, c0 + h:c0 + CHUNK])
                else:
                    nc.sync.dma_start(out=xc[:], in_=x_d[:, c0:c0 + CHUNK])
                chunk_tiles[ci] = xc

            def evac(pshalf, htile, half, bias_col, dve_cols):
                """relu+bias PSUM->SBUF; column-split ACT/DVE."""
                ac = NTILE - dve_cols
                if dve_cols > 0:
                    eng = nc.gpsimd if POOL_EVAC else nc.vector
                    eng.tensor_scalar(
                        out=htile[:, 2 * half:2 * half + 2, ac:NTILE],
                        in0=pshalf[:, :, ac:NTILE],
                        scalar1=bs[:, bias_col:bias_col + 1],
                        scalar2=0.0, op0=ALU.add, op1=ALU.max)
                if ac > 0:
                    nc.scalar.activation(
                        out=htile[:, 2 * half:2 * half + 2, 0:ac],
                        in_=pshalf[:, :, 0:ac], func=AF.Relu,
                        bias=bs[:, bias_col:bias_col + 1], scale=1.0)

            def stage1(d):
                """L1 matmuls + evac1 -> h1."""
                ci = d["chunk"]
                while issued[0] <= min(ci + 1, len(chunks) - 1):
                    issue_chunk(issued[0])
                    issued[0] += 1
                xc = chunk_tiles[ci]
                h1 = hbuf.tile([128, 4, NTILE], MMDT, tag="h1")
                d["h1"] = h1
                pstiles = []
                for half in range(2):
                    p = ps.tile([128, 2, NTILE], F32, tag="ps")
                    pstiles.append(p)
                    for j in range(2):
                        q = 2 * half + j
                        nc.tensor.matmul(
                            p[:, j, :],
                            wts[32 * q:32 * q + d["k2"],
                                d["w1_c"]:d["w1_c"] + 128],
                            xc[32 * q:32 * q + d["k2"], d["cs"]],
                            start=True, stop=True,
                            tile_position=(32 * q, 0))
                for half in range(2):
                    evac(pstiles[half], h1, half, d["bs1"], SPLITS[half])

            def stage2(d):
                """L2 matmuls + evac2 -> h2."""
                h1 = d.pop("h1")
                h2 = hbuf.tile([128, 4, NTILE], BF16, tag="h2")
                d["h2"] = h2
                pstiles = []
                for half in range(2):
                    p = ps.tile([128, 2, NTILE], F32, tag="ps")
                    pstiles.append(p)
                    for j in range(2):
                        q = 2 * half + j
                        nc.tensor.matmul(
                            p[:, j, :],
                            wts[:, d["w2_c"]:d["w2_c"] + 128],
                            h1[:, q, :], start=True, stop=True)
                for half in range(2):
                    evac(pstiles[half], h2, half, d["bs2"], SPLITS[2 + half])

            def stage3(d):
                """L3 (col-paired) + pooling reduces."""
                h2 = d.pop("h2")
                nb, nseg = d["nb"], d["nseg"]
                pool3 = ps3p if PS3_BUFS > 0 else ps
                ps3 = pool3.tile([128, 2 * nb * nseg], F32,
                                 tag="ps3" if PS3_BUFS else "ps")
                for q in range(4):
                    J, blk = q // 2, q % 2
                    nc.tensor.matmul(
                        ps3[64 * blk:64 * blk + 64,
                            J * NTILE:(J + 1) * NTILE],
                        wbf[:, d["w3_c"]:d["w3_c"] + 64],
                        h2[:, q, :], start=True, stop=True,
                        tile_position=(0, 64 * blk))
                p3r = ps3.rearrange("p (a b c) -> p (a b) c", a=2, b=nb)
                base = d["base"]
                sview = d["esum"].rearrange("p (J r) -> p J r", J=2)[
                    :, :, base:base + nb]
                mview = d["emax"].rearrange("p (J r) -> p J r", J=2)[
                    :, :, base:base + nb]
                if not PROBE_NO_REDUCE:
                    nc.vector.reduce_sum(out=sview, in_=p3r[:], axis=AX.X)
                    nc.vector.reduce_max(out=mview, in_=p3r[:], axis=AX.X)

            emb_t = acc.tile([128, 512], MMDT, tag="emb_t")
            emb_r = acc.tile([128, 512], MMDT, tag="emb_r")

            def combine_and_scatter(esum, emax, emb, nseg, row0, p0, p1,
                                    parallel_q=False):
                """emb = esum/(2*nseg) + 0.5*emax for pair-cols [p0, p1) of
                each J half, then scatter into cT. Split so most of it can
                issue before a branch's last reduce."""
                npair = p1 - p0
                eview = lambda t: t.rearrange("p (J x) -> p J x", J=2)[
                    :, :, p0:p1]
                tmp = hbuf.tile([128, 512], F32, tag="tmp")
                tv = tmp.rearrange("p (J x) -> p J x", J=2)[:, :, p0:p1]
                nc.vector.tensor_scalar(out=tv, in0=eview(esum),
                                        scalar1=1.0 / (2.0 * nseg),
                                        scalar2=None, op0=ALU.mult)
                nc.vector.scalar_tensor_tensor(
                    out=eview(emb), in0=eview(emax), scalar=0.5, in1=tv,
                    op0=ALU.mult, op1=ALU.add)
                # scatter, J-dim merged: 4 DMAs per call
                embJ = emb.rearrange("p (J x) -> p J x", J=2)[:, :, p0:p1]
                cTv = cT[row0:row0 + 32, :].rearrange(
                    "p (h J blk x) -> p h J blk x", h=2, J=2, blk=2)
                engs = ([nc.sync, nc.scalar, nc.gpsimd, nc.sync]
                        if parallel_q else [nc.sync] * 4)
                for blk in range(2):
                    for h in range(2):
                        engs[2 * blk + h].dma_start(
                            out=cTv[:, h, :, blk, p0:p1],
                            in_=embJ[64 * blk + 32 * h:
                                     64 * blk + 32 * h + 32, :, :])

            n = len(descs)
            ntrack = QT // NTILE  # first ntrack descs are the track branch
            # prime the pipeline: x chunks first, then non-critical consts
            issue_chunk(0)
            issue_chunk(1)
            issued[0] = 2
            nc.sync.dma_start(out=wts[:, 256:WTS_W], in_=wts_d[:, 256:WTS_W])
            nc.sync.dma_start(out=wbf[:], in_=wbf_d[:])
            nc.sync.dma_start(out=cT[0:44, :], in_=t0_d[:])
            for i in range(n + 2):
                if i < n:
                    stage1(descs[i])
                if 1 <= i <= n:
                    stage2(descs[i - 1])
                if 2 <= i <= n + 1:
                    stage3(descs[i - 2])
                if i - 2 == ntrack - 1:
                    combine_and_scatter(esum_t, emax_t, emb_t, NT, 76,
                                        0, 256)
                if i - 2 == ntrack + 23:
                    # robot pair-cols [0,192) are final after robot desc 23
                    combine_and_scatter(esum_r, emax_r, emb_r, NR, 44,
                                        0, 192)
            combine_and_scatter(esum_r, emax_r, emb_r, NR, 44, 192, 256,
                                parallel_q=True)

            # keep the PE p-state warm across the drain->head gap with
            # dummy matmuls on resident constants (results never read)
            NWARM = int(os.environ.get("NWARM", "0"))
            if NWARM:
                warm = ps.tile([128, 2, NTILE], F32, tag="ps")
                for w in range(NWARM):
                    nc.tensor.matmul(warm[:, w % 2, :], wts[:, 0:128],
                                     wts[:, 256:768], start=True, stop=True)

            # ---- head MLP 108 -> 128 -> 128 -> 64 -> 1, layer-major ----
            # evacs alternate ACT/DVE so the 4 col-tiles pipeline.
            y_sb = acc.tile([1, B_C], F32, tag="y")
            NHT = B_C // NTILE  # 4 col-tiles

            def head_evac(pst, dsts, s, bias_col, prange=128, func=AF.Relu):
                if s % 2 == 0:
                    nc.scalar.activation(
                        out=dsts, in_=pst[:], func=func,
                        bias=bs[0:prange, bias_col:bias_col + 1], scale=1.0)
                elif func == AF.Relu:
                    nc.vector.tensor_scalar(
                        out=dsts, in0=pst[:],
                        scalar1=bs[0:prange, bias_col:bias_col + 1],
                        scalar2=0.0, op0=ALU.add, op1=ALU.max)
                else:
                    nc.vector.tensor_scalar(
                        out=dsts, in0=pst[:],
                        scalar1=bs[0:prange, bias_col:bias_col + 1],
                        scalar2=None, op0=ALU.add)

            hh1 = head.tile([128, B_C], MMDT, tag="hh1")
            hh2 = head.tile([128, B_C], MMDT, tag="hh2")
            hh3 = head.tile([64, B_C], MMDT, tag="hh3")
            layers = (
                (lambda s: wts[0:108, MW1_C:MW1_C + 128],
                 lambda s: cT[:, s * NTILE:(s + 1) * NTILE],
                 hh1, BS_MB1, 128),
                (lambda s: wts[:, MW2_C:MW2_C + 128],
                 lambda s: hh1[:, s * NTILE:(s + 1) * NTILE],
                 hh2, BS_MB2, 128),
                (lambda s: wts[:, MW3_C:MW3_C + 64],
                 lambda s: hh2[:, s * NTILE:(s + 1) * NTILE],
                 hh3, BS_MB3, 64),
            )
            for wfn, infn, dst, bcol, prange in layers:
                psl = []
                for s in range(NHT):
                    p = ps.tile([prange, NTILE], F32, tag="ps")
                    psl.append(p)
                    nc.tensor.matmul(p[:], wfn(s), infn(s),
                                     start=True, stop=True)
                for s in range(NHT):
                    head_evac(psl[s],
                              dst[0:prange, s * NTILE:(s + 1) * NTILE],
                              s, bcol, prange)
            psl = []
            for s in range(NHT):
                p = ps.tile([1, NTILE], F32, tag="ps")
                psl.append(p)
                nc.tensor.matmul(p[:], wts[0:64, MW4_C:MW4_C + 1],
                                 hh3[:, s * NTILE:(s + 1) * NTILE],
                                 start=True, stop=True)
            for s in range(NHT):
                head_evac(psl[s], y_sb[:, s * NTILE:(s + 1) * NTILE],
                          s, BS_MB4, 1, AF.Identity)
            nc.sync.dma_start(out=y_d[:], in_=y_sb[:])

    nc.compile()
    return nc


def _pack_x(x, d, qcols):
    """x [rows, d] (rows = B_c*nseg, b-major) -> [128, qcols] with 4
    row-groups at partition offsets {0,32,64,96}; 2-row packing pairs
    row r with row r + rows/2."""
    rows = x.shape[0]
    half = rows // 2
    packed = np.concatenate([x[:half].T, x[half:].T], axis=0)  # [2d, half]
    out = np.zeros((128, qcols), dtype=MMDT_NP)
    for q in range(4):
        out[32 * q:32 * q + 2 * d] = packed[:, q * qcols:(q + 1) * qcols]
    return np.ascontiguousarray(out)


def _blockdiag2(w):
    """w [d, m] -> [2d, 2m] block-diagonal."""
    d, m = w.shape
    out = np.zeros((2 * d, 2 * m), dtype=np.float32)
    out[:d, :m] = w
    out[d:, m:] = w
    return out


def _build_consts(i):
    np32 = lambda a: np.asarray(a, dtype=np.float32)
    wts = np.zeros((128, WTS_W), dtype=np.float32)
    # L1 lhsT blocks replicated at the 4 row-group offsets
    bd1t = _blockdiag2(np32(i["tw1"]))   # [14, 128]
    bd1r = _blockdiag2(np32(i["rw1"]))   # [12, 128]
    for q in range(4):
        wts[32 * q:32 * q + 14, W1T_C:W1T_C + 128] = bd1t
        wts[32 * q:32 * q + 12, W1R_C:W1R_C + 128] = bd1r
    wts[:, W2T_C:W2T_C + 128] = _blockdiag2(np32(i["tw2"]))
    wts[:, W2R_C:W2R_C + 128] = _blockdiag2(np32(i["rw2"]))
    wts[0:108, MW1_C:MW1_C + 128] = np32(i["mw1"])
    wts[:, MW2_C:MW2_C + 128] = np32(i["mw2"])
    wts[:, MW3_C:MW3_C + 64] = np32(i["mw3"])
    wts[0:64, MW4_C:MW4_C + 1] = np32(i["mw4"])
    wts = wts.astype(MMDT_NP)

    wbf = np.zeros((128, 128), dtype=np.float32)
    wbf[:, 0:64] = _blockdiag2(np32(i["tw3"]))
    wbf[:, 64:128] = _blockdiag2(np32(i["rw3"]))
    wbf = wbf.astype(ml_dtypes.bfloat16)

    bs = np.zeros((128, 8), dtype=np.float32)
    bs[:, BS_TB1] = np.concatenate([np32(i["tb1"]), np32(i["tb1"])])
    bs[:, BS_RB1] = np.concatenate([np32(i["rb1"]), np32(i["rb1"])])
    bs[:, BS_TB2] = np.concatenate([np32(i["tb2"]), np32(i["tb2"])])
    bs[:, BS_RB2] = np.concatenate([np32(i["rb2"]), np32(i["rb2"])])
    # fold pooled e-bias into head L1 bias: c@mw1 picks up b3@mw1 rows
    mb1p = (np32(i["mb1"])
            + np32(i["rb3"]) @ np32(i["mw1"])[44:76]
            + np32(i["tb3"]) @ np32(i["mw1"])[76:108])
    bs[:, BS_MB1] = mb1p
    bs[:, BS_MB2] = np32(i["mb2"])
    bs[0:64, BS_MB3] = np32(i["mb3"])
    bs[0:1, BS_MB4] = np32(i["mb4"])
    return wts, wbf, bs


def kernel(**inputs) -> np.ndarray:
    if "nc" not in _CACHE:
        _CACHE["nc"] = _build_bass()
    nc = _CACHE["nc"]

    wts, wbf, bs = _build_consts(inputs)
    t0 = np.asarray(inputs["tier0_features"], dtype=np.float32)
    rb = np.asarray(inputs["robot_features"], dtype=np.float32)
    tk = np.asarray(inputs["track_features"], dtype=np.float32)

    in_maps = []
    for c in range(N_CORES):
        s = slice(c * B_C, (c + 1) * B_C)
        in_maps.append({
            "xt": _pack_x(tk[s].reshape(B_C * NT, DT), DT, QT),
            "xr": _pack_x(rb[s].reshape(B_C * NR, DR), DR, QR),
            "t0": np.ascontiguousarray(t0[s].T).astype(MMDT_NP),
            "wts": wts, "wbf": wbf, "bs": bs,
        })

    res = run_bass_kernel_spmd(nc, in_maps, core_ids=list(range(N_CORES)))
    out = np.concatenate([r["y"][0] for r in res.results])
    return out.astype(np.float32)


if __name__ == "__main__":
    rng = np.random.default_rng(0)
    fake = {
        "tier0_features": rng.standard_normal((B, 44), dtype=np.float32),
        "robot_features": rng.standard_normal((B, NR, DR), dtype=np.float32),
        "track_features": rng.standard_normal((B, NT, DT), dtype=np.float32),
    }
    for n, sh in (("rw1", (6, 64)), ("rw2", (64, 64)), ("rw3", (64, 32)),
                  ("tw1", (7, 64)), ("tw2", (64, 64)), ("tw3", (64, 32)),
                  ("mw1", (108, 128)), ("mw2", (128, 128)),
                  ("mw3", (128, 64)), ("mw4", (64, 1))):
        fake[n] = rng.standard_normal(sh, dtype=np.float32) * 0.2
    for n, sh in (("rb1", 64), ("rb2", 64), ("rb3", 32),
                  ("tb1", 64), ("tb2", 64), ("tb3", 32),
                  ("mb1", 128), ("mb2", 128), ("mb3", 64), ("mb4", 1)):
        fake[n] = rng.standard_normal((sh,), dtype=np.float32) * 0.1
    y = kernel(**fake)
    print("kernel out:", y.shape, y[:4])



# revision 37
# speedup vs baseline: 1.0019x; 1.0001x over previous
"""Trainium2 Bass kernel for nn_CentralizedCritic (pooling critic net).

Data-parallel over 8 NeuronCores: each core handles B_c=2048 batch rows.

Per-core math (matches the jax reference):
  robot_emb = setenc(robot[b], rw*)  -> [B,32]   (mean+max pool over 64)
  track_emb = setenc(track[b], tw*)  -> [B,32]   (mean+max pool over 128)
  c = [tier0, robot_emb, track_emb]  -> [B,108]
  y = mlp(c)                         -> [B]

On-chip mapping:
  - Activations kept transposed [feat, rows]; 2 batch-halves packed on the
    partition dim via block-diag weights (K=2*d_in, M=2*d_hidden=128).
  - x^T pre-packed on host into 4 row-groups at partition offsets {0,32,64,96}
    so L1 matmuls row-tile the PE array. bf16 throughout (rel err ~7e-3).
  - Both branches flattened into one desc list and software-pipelined 3 deep
    (PE per iter: L1(i), L2(i-1), L3(i-2)) so the PE never waits on a
    same-iteration evacuation.
  - relu+bias fused into the PSUM->SBUF evacuation; evacs column-split
    between ACT and DVE (SPLITS) so ACT-busy ~= DVE-busy (reduces are
    DVE-only: ACT and DVE are the only PSUM-capable engines, which sets
    the ~3.6us/tile steady-state floor).
  - mean-pool: DVE reduce_sum from PSUM; max-pool: DVE reduce_max from PSUM.
  - e-bias (rb3/tb3) folded into the head-L1 bias on host.
  - branch combine+scatter issued as soon as its pair-columns are final
    (track fully + robot cols [0,192) hidden under the steady state); head
    runs layer-major with ACT/DVE-alternating evacs so its 4 col-tiles
    pipeline.
"""

import sys

sys.path.insert(0, "/opt/trn_rl_repo")

import numpy as np
import ml_dtypes

import concourse.bass as bass  # noqa: F401  (bass must import before tile)
import concourse.mybir as mybir
import concourse.tile as tile
from concourse import bacc
from concourse.bass_utils import run_bass_kernel_spmd

F32R = mybir.dt.float32r
F32 = mybir.dt.float32
BF16 = mybir.dt.bfloat16
AF = mybir.ActivationFunctionType
ALU = mybir.AluOpType
AX = mybir.AxisListType

N_CORES = 8
B = 16384
B_C = B // N_CORES          # 2048 batch rows per core
HALF = B_C // 2             # 1024 (2-row packing pairs b and b+HALF)
NR, DR = 64, 6              # robot set size / feature dim
NT, DT = 128, 7             # track set size / feature dim
CT = B_C * NT // 2          # 131072 packed track cols per core
CR = B_C * NR // 2          # 65536 packed robot cols per core
QT = CT // 4                # 32768 cols per track row-group
QR = CR // 4                # 16384 cols per robot row-group
CHUNK = int(__import__("os").environ.get("CHUNK", "2048"))  # dma chunk cols
NTILE = 512                 # matmul free dim
# DVE column-split per evac slot [e1h0, e1h1, e2h0, e2h1]: how many of the
# NTILE columns of that evac go to DVE (rest on ACT). 0=all ACT, 512=all DVE.
SPLITS = tuple(int(x) for x in
               __import__("os").environ.get("SPLITS", "0,0,32,320").split(","))
LOOKAHEAD = int(__import__("os").environ.get("LOOKAHEAD", "5"))

# const-block column layout in "wts" [128, 840] (f32r)
W1T_C, W1R_C, W2T_C, W2R_C = 0, 128, 256, 384
MW1_C, MW2_C, MW3_C, MW4_C = 512, 640, 768, 832
WTS_W = 840
# "bs" [128, 8] (f32) bias columns
BS_TB1, BS_RB1, BS_TB2, BS_RB2, BS_MB1, BS_MB2, BS_MB3, BS_MB4 = range(8)

_CACHE = {}

import os
PROBE_NO_REDUCE = os.environ.get("PROBE_NO_REDUCE") == "1"
POOL_EVAC = os.environ.get("POOL_EVAC", "0") == "1"
PROBE_NO_EVAC = os.environ.get("PROBE_NO_EVAC") == "1"
PROBE_NO_MM3 = os.environ.get("PROBE_NO_MM3") == "1"
HBUF_BUFS = int(os.environ.get("HBUF_BUFS", "3"))
XC_BUFS = int(os.environ.get("XC_BUFS", "4"))
PS_BUFS = int(os.environ.get("PS_BUFS", "3"))
PS3_BUFS = int(os.environ.get("PS3_BUFS", "1"))  # 0 = share "ps" pool
DVE_EVAC_MOD = int(os.environ.get("DVE_EVAC_MOD", "3"))
MMDT_NAME = os.environ.get("MMDT", "bf16")
SPLIT_L3 = os.environ.get("SPLIT_L3", "0") == "1"
EVAC_ASSIGN = os.environ.get("EVAC_ASSIGN", "rr")  # rr | e1
MMDT = mybir.dt.bfloat16 if MMDT_NAME == "bf16" else mybir.dt.float32r
MMDT_NP = ml_dtypes.bfloat16 if MMDT_NAME == "bf16" else np.float32


def _build_bass():
    nc = bacc.Bacc("TRN2", target_bir_lowering=False, debug=False,
                   num_devices=N_CORES)
    xt_d = nc.dram_tensor("xt", [128, QT], MMDT, kind="ExternalInput")
    xr_d = nc.dram_tensor("xr", [128, QR], MMDT, kind="ExternalInput")
    t0_d = nc.dram_tensor("t0", [44, B_C], MMDT, kind="ExternalInput")
    wts_d = nc.dram_tensor("wts", [128, WTS_W], MMDT, kind="ExternalInput")
    wbf_d = nc.dram_tensor("wbf", [128, 128], BF16, kind="ExternalInput")
    bs_d = nc.dram_tensor("bs", [128, 8], F32, kind="ExternalInput")
    y_d = nc.dram_tensor("y", [1, B_C], F32, kind="ExternalOutput")

    with tile.TileContext(nc) as tc:
        with (
            tc.tile_pool(name="consts", bufs=1) as consts,
            tc.tile_pool(name="xchunks", bufs=XC_BUFS) as xchunks,
            tc.tile_pool(name="hbuf", bufs=HBUF_BUFS) as hbuf,
            tc.tile_pool(name="acc", bufs=1) as acc,
            tc.tile_pool(name="head", bufs=2) as head,
            tc.tile_pool(name="ps", bufs=PS_BUFS, space="PSUM") as ps,
            tc.tile_pool(name="ps3p", bufs=max(PS3_BUFS, 1),
                         space="PSUM") as ps3p,
        ):
            wts = consts.tile([128, WTS_W], MMDT)
            wbf = consts.tile([128, 128], BF16)
            bs = consts.tile([128, 8], F32)
            nc.sync.dma_start(out=wts[:, 0:128], in_=wts_d[:, 0:128])
            nc.sync.dma_start(out=wts[:, 128:256], in_=wts_d[:, 128:256])
            nc.sync.dma_start(out=bs[:], in_=bs_d[:])

            cT = acc.tile([108, B_C], MMDT)

            # pooled accumulators: col = 256*J + 32*ch + 4*t + om (track)
            #                      col = 256*J + 64*ch + 8*t + om (robot)
            esum_t = acc.tile([128, 512], F32, tag="esum_t")
            emax_t = acc.tile([128, 512], F32, tag="emax_t")
            esum_r = acc.tile([128, 512], F32, tag="esum_r")
            emax_r = acc.tile([128, 512], F32, tag="emax_r")

            # ---- unified descriptor list over both branches ----
            # each desc = one NTILE-wide tile step of a branch
            descs = []
            chunks = []  # (x_d, col0, first_desc_idx)
            for (x_d, qcols, k2, w1_c, w2_c, w3_c, bs1, bs2, nseg,
                 esum, emax) in (
                    (xt_d, QT, 2 * DT, W1T_C, W2T_C, 0, BS_TB1, BS_TB2, NT,
                     esum_t, emax_t),
                    (xr_d, QR, 2 * DR, W1R_C, W2R_C, 64, BS_RB1, BS_RB2, NR,
                     esum_r, emax_r)):
                nchunks = qcols // CHUNK
                tpc = CHUNK // NTILE
                tg = 0
                for ch in range(nchunks):
                    chunks.append((x_d, ch * CHUNK, len(descs)))
                    for t in range(tpc):
                        descs.append(dict(
                            chunk=len(chunks) - 1, cs=slice(t * NTILE,
                                                            (t + 1) * NTILE),
                            k2=k2, w1_c=w1_c, w2_c=w2_c, w3_c=w3_c,
                            bs1=bs1, bs2=bs2, nseg=nseg, esum=esum,
                            emax=emax, nb=NTILE // nseg,
                            base=(NTILE // nseg) * tg))
                        tg += 1

            chunk_tiles = {}
            issued = [0]  # chunks issued so far

            def issue_chunk(ci):
                x_d, c0, _ = chunks[ci]
                xc = xchunks.tile([128, CHUNK], MMDT, tag="xc")
                if ci == 0:
                    # split the first chunk so desc 0 unblocks after the
                    # first half lands (range-level dep tracking)
                    h = CHUNK // 2
                    nc.sync.dma_start(out=xc[:, 0:h], in_=x_d[:, c0:c0 + h])
                    nc.sync.dma_start(out=xc[:, h:CHUNK],
                                      in_=x_d[:, c0 + h:c0 + CHUNK])
                else:
                    nc.sync.dma_start(out=xc[:], in_=x_d[:, c0:c0 + CHUNK])
                chunk_tiles[ci] = xc

            def evac(pshalf, htile, half, bias_col, dve_cols):
                """relu+bias PSUM->SBUF; column-split ACT/DVE."""
                ac = NTILE - dve_cols
                if dve_cols > 0:
                    eng = nc.gpsimd if POOL_EVAC else nc.vector
                    eng.tensor_scalar(
                        out=htile[:, 2 * half:2 * half + 2, ac:NTILE],
                        in0=pshalf[:, :, ac:NTILE],
                        scalar1=bs[:, bias_col:bias_col + 1],
                        scalar2=0.0, op0=ALU.add, op1=ALU.max)
                if ac > 0:
                    nc.scalar.activation(
                        out=htile[:, 2 * half:2 * half + 2, 0:ac],
                        in_=pshalf[:, :, 0:ac], func=AF.Relu,
                        bias=bs[:, bias_col:bias_col + 1], scale=1.0)

            def stage1(d):
                """L1 matmuls + evac1 -> h1."""
                ci = d["chunk"]
                while issued[0] <= min(ci + 1, len(chunks) - 1):
                    issue_chunk(issued[0])
                    issued[0] += 1
                xc = chunk_tiles[ci]
                h1 = hbuf.tile([128, 4, NTILE], MMDT, tag="h1")
                d["h1"] = h1
                pstiles = []
                for half in range(2):
                    p = ps.tile([128, 2, NTILE], F32, tag="ps")
                    pstiles.append(p)
                    for j in range(2):
                        q = 2 * half + j
                        nc.tensor.matmul(
                            p[:, j, :],
                            wts[32 * q:32 * q + d["k2"],
                                d["w1_c"]:d["w1_c"] + 128],
                            xc[32 * q:32 * q + d["k2"], d["cs"]],
                            start=True, stop=True,
                            tile_position=(32 * q, 0))
                for half in range(2):
                    evac(pstiles[half], h1, half, d["bs1"], SPLITS[half])

            def stage2(d):
                """L2 matmuls + evac2 -> h2."""
                h1 = d.pop("h1")
                h2 = hbuf.tile([128, 4, NTILE], BF16, tag="h2")
                d["h2"] = h2
                pstiles = []
                for half in range(2):
                    p = ps.tile([128, 2, NTILE], F32, tag="ps")
                    pstiles.append(p)
                    for j in range(2):
                        q = 2 * half + j
                        nc.tensor.matmul(
                            p[:, j, :],
                            wts[:, d["w2_c"]:d["w2_c"] + 128],
                            h1[:, q, :], start=True, stop=True)
                for half in range(2):
                    evac(pstiles[half], h2, half, d["bs2"], SPLITS[2 + half])

            def stage3(d):
                """L3 (col-paired) + pooling reduces."""
                h2 = d.pop("h2")
                nb, nseg = d["nb"], d["nseg"]
                pool3 = ps3p if PS3_BUFS > 0 else ps
                ps3 = pool3.tile([128, 2 * nb * nseg], F32,
                                 tag="ps3" if PS3_BUFS else "ps")
                for q in range(4):
                    J, blk = q // 2, q % 2
                    nc.tensor.matmul(
                        ps3[64 * blk:64 * blk + 64,
                            J * NTILE:(J + 1) * NTILE],
                        wbf[:, d["w3_c"]:d["w3_c"] + 64],
                        h2[:, q, :], start=True, stop=True,
                        tile_position=(0, 64 * blk))
                p3r = ps3.rearrange("p (a b c) -> p (a b) c", a=2, b=nb)
                base = d["base"]
                sview = d["esum"].rearrange("p (J r) -> p J r", J=2)[
                    :, :, base:base + nb]
                mview = d["emax"].rearrange("p (J r) -> p J r", J=2)[
                    :, :, base:base + nb]
                if not PROBE_NO_REDUCE:
                    nc.vector.reduce_sum(out=sview, in_=p3r[:], axis=AX.X)
                    nc.vector.reduce_max(out=mview, in_=p3r[:], axis=AX.X)

            emb_t = acc.tile([128, 512], MMDT, tag="emb_t")
            emb_r = acc.tile([128, 512], MMDT, tag="emb_r")

            def combine_and_scatter(esum, emax, emb, nseg, row0, p0, p1,
                                    parallel_q=False):
                """emb = esum/(2*nseg) + 0.5*emax for pair-cols [p0, p1) of
                each J half, then scatter into cT. Split so most of it can
                issue before a branch's last reduce."""
                npair = p1 - p0
                eview = lambda t: t.rearrange("p (J x) -> p J x", J=2)[
                    :, :, p0:p1]
                tmp = hbuf.tile([128, 512], F32, tag="tmp")
                tv = tmp.rearrange("p (J x) -> p J x", J=2)[:, :, p0:p1]
                nc.vector.tensor_scalar(out=tv, in0=eview(esum),
                                        scalar1=1.0 / (2.0 * nseg),
                                        scalar2=None, op0=ALU.mult)
                nc.vector.scalar_tensor_tensor(
                    out=eview(emb), in0=eview(emax), scalar=0.5, in1=tv,
                    op0=ALU.mult, op1=ALU.add)
                # scatter, J-dim merged: 4 DMAs per call
                embJ = emb.rearrange("p (J x) -> p J x", J=2)[:, :, p0:p1]
                cTv = cT[row0:row0 + 32, :].rearrange(
                    "p (h J blk x) -> p h J blk x", h=2, J=2, blk=2)
                engs = ([nc.sync, nc.scalar, nc.gpsimd, nc.sync]
                        if parallel_q else [nc.sync] * 4)
                for blk in range(2):
                    for h in range(2):
                        engs[2 * blk + h].dma_start(
                            out=cTv[:, h, :, blk, p0:p1],
                            in_=embJ[64 * blk + 32 * h:
                                     64 * blk + 32 * h + 32, :, :])

            n = len(descs)
            ntrack = QT // NTILE  # first ntrack descs are the track branch
            # prime the pipeline: x chunks first, then non-critical consts
            issue_chunk(0)
            issue_chunk(1)
            issued[0] = 2
            nc.sync.dma_start(out=wts[:, 256:WTS_W], in_=wts_d[:, 256:WTS_W])
            nc.sync.dma_start(out=wbf[:], in_=wbf_d[:])
            nc.sync.dma_start(out=cT[0:44, :], in_=t0_d[:])
            for i in range(n + 2):
                if i < n:
                    stage1(descs[i])
                if 1 <= i <= n:
                    stage2(descs[i - 1])
                if 2 <= i <= n + 1:
                    stage3(descs[i - 2])
                if i - 2 == ntrack - 1:
                    combine_and_scatter(esum_t, emax_t, emb_t, NT, 76,
                                        0, 256)
                if i - 2 == ntrack + 23:
                    # robot pair-cols [0,192) are final after robot desc 23
                    combine_and_scatter(esum_r, emax_r, emb_r, NR, 44,
                                        0, 192)
                if i - 2 == ntrack + 30:
                    combine_and_scatter(esum_r, emax_r, emb_r, NR, 44,
                                        192, 248)
            combine_and_scatter(esum_r, emax_r, emb_r, NR, 44, 248, 256,
                                parallel_q=True)

            # keep the PE p-state warm across the drain->head gap with
            # dummy matmuls on resident constants (results never read)
            NWARM = int(os.environ.get("NWARM", "0"))
            if NWARM:
                warm = ps.tile([128, 2, NTILE], F32, tag="ps")
                for w in range(NWARM):
                    nc.tensor.matmul(warm[:, w % 2, :], wts[:, 0:128],
                                     wts[:, 256:768], start=True, stop=True)

            # ---- head MLP 108 -> 128 -> 128 -> 64 -> 1, layer-major ----
            # evacs alternate ACT/DVE so the 4 col-tiles pipeline.
            y_sb = acc.tile([1, B_C], F32, tag="y")
            NHT = B_C // NTILE  # 4 col-tiles

            def head_evac(pst, dsts, s, bias_col, prange=128, func=AF.Relu):
                if s % 2 == 0:
                    nc.scalar.activation(
                        out=dsts, in_=pst[:], func=func,
                        bias=bs[0:prange, bias_col:bias_col + 1], scale=1.0)
                elif func == AF.Relu:
                    nc.vector.tensor_scalar(
                        out=dsts, in0=pst[:],
                        scalar1=bs[0:prange, bias_col:bias_col + 1],
                        scalar2=0.0, op0=ALU.add, op1=ALU.max)
                else:
                    nc.vector.tensor_scalar(
                        out=dsts, in0=pst[:],
                        scalar1=bs[0:prange, bias_col:bias_col + 1],
                        scalar2=None, op0=ALU.add)

            hh1 = head.tile([128, B_C], MMDT, tag="hh1")
            hh2 = head.tile([128, B_C], MMDT, tag="hh2")
            hh3 = head.tile([64, B_C], MMDT, tag="hh3")
            layers = (
                (lambda s: wts[0:108, MW1_C:MW1_C + 128],
                 lambda s: cT[:, s * NTILE:(s + 1) * NTILE],
                 hh1, BS_MB1, 128),
                (lambda s: wts[:, MW2_C:MW2_C + 128],
                 lambda s: hh1[:, s * NTILE:(s + 1) * NTILE],
                 hh2, BS_MB2, 128),
                (lambda s: wts[:, MW3_C:MW3_C + 64],
                 lambda s: hh2[:, s * NTILE:(s + 1) * NTILE],
                 hh3, BS_MB3, 64),
            )
            for wfn, infn, dst, bcol, prange in layers:
                psl = []
                for s in range(NHT):
                    p = ps.tile([prange, NTILE], F32, tag="ps")
                    psl.append(p)
                    nc.tensor.matmul(p[:], wfn(s), infn(s),
                                     start=True, stop=True)
                for s in range(NHT):
                    head_evac(psl[s],
                              dst[0:prange, s * NTILE:(s + 1) * NTILE],
                              s, bcol, prange)
            psl = []
            for s in range(NHT):
                p = ps.tile([1, NTILE], F32, tag="ps")
                psl.append(p)
                nc.tensor.matmul(p[:], wts[0:64, MW4_C:MW4_C + 1],
                                 hh3[:, s * NTILE:(s + 1) * NTILE],
                                 start=True, stop=True)
            for s in range(NHT):
                head_evac(psl[s], y_sb[:, s * NTILE:(s + 1) * NTILE],
                          s, BS_MB4, 1, AF.Identity)
            nc.sync.dma_start(out=y_d[:, 0:B_C // 2],
                              in_=y_sb[:, 0:B_C // 2])
            nc.sync.dma_start(out=y_d[:, B_C // 2:B_C],
                              in_=y_sb[:, B_C // 2:B_C])

    nc.compile()
    return nc


def _pack_x(x, d, qcols):
    """x [rows, d] (rows = B_c*nseg, b-major) -> [128, qcols] with 4
    row-groups at partition offsets {0,32,64,96}; 2-row packing pairs
    row r with row r + rows/2."""
    rows = x.shape[0]
    half = rows // 2
    packed = np.concatenate([x[:half].T, x[half:].T], axis=0)  # [2d, half]
    out = np.zeros((128, qcols), dtype=MMDT_NP)
    for q in range(4):
        out[32 * q:32 * q + 2 * d] = packed[:, q * qcols:(q + 1) * qcols]
    return np.ascontiguousarray(out)


def _blockdiag2(w):
    """w [d, m] -> [2d, 2m] block-diagonal."""
    d, m = w.shape
    out = np.zeros((2 * d, 2 * m), dtype=np.float32)
    out[:d, :m] = w
    out[d:, m:] = w
    return out


def _build_consts(i):
    np32 = lambda a: np.asarray(a, dtype=np.float32)
    wts = np.zeros((128, WTS_W), dtype=np.float32)
    # L1 lhsT blocks replicated at the 4 row-group offsets
    bd1t = _blockdiag2(np32(i["tw1"]))   # [14, 128]
    bd1r = _blockdiag2(np32(i["rw1"]))   # [12, 128]
    for q in range(4):
        wts[32 * q:32 * q + 14, W1T_C:W1T_C + 128] = bd1t
        wts[32 * q:32 * q + 12, W1R_C:W1R_C + 128] = bd1r
    wts[:, W2T_C:W2T_C + 128] = _blockdiag2(np32(i["tw2"]))
    wts[:, W2R_C:W2R_C + 128] = _blockdiag2(np32(i["rw2"]))
    wts[0:108, MW1_C:MW1_C + 128] = np32(i["mw1"])
    wts[:, MW2_C:MW2_C + 128] = np32(i["mw2"])
    wts[:, MW3_C:MW3_C + 64] = np32(i["mw3"])
    wts[0:64, MW4_C:MW4_C + 1] = np32(i["mw4"])
    wts = wts.astype(MMDT_NP)

    wbf = np.zeros((128, 128), dtype=np.float32)
    wbf[:, 0:64] = _blockdiag2(np32(i["tw3"]))
    wbf[:, 64:128] = _blockdiag2(np32(i["rw3"]))
    wbf = wbf.astype(ml_dtypes.bfloat16)

    bs = np.zeros((128, 8), dtype=np.float32)
    bs[:, BS_TB1] = np.concatenate([np32(i["tb1"]), np32(i["tb1"])])
    bs[:, BS_RB1] = np.concatenate([np32(i["rb1"]), np32(i["rb1"])])
    bs[:, BS_TB2] = np.concatenate([np32(i["tb2"]), np32(i["tb2"])])
    bs[:, BS_RB2] = np.concatenate([np32(i["rb2"]), np32(i["rb2"])])
    # fold pooled e-bias into head L1 bias: c@mw1 picks up b3@mw1 rows
    mb1p = (np32(i["mb1"])
            + np32(i["rb3"]) @ np32(i["mw1"])[44:76]
            + np32(i["tb3"]) @ np32(i["mw1"])[76:108])
    bs[:, BS_MB1] = mb1p
    bs[:, BS_MB2] = np32(i["mb2"])
    bs[0:64, BS_MB3] = np32(i["mb3"])
    bs[0:1, BS_MB4] = np32(i["mb4"])
    return wts, wbf, bs


def kernel(**inputs) -> np.ndarray:
    if "nc" not in _CACHE:
        _CACHE["nc"] = _build_bass()
    nc = _CACHE["nc"]

    wts, wbf, bs = _build_consts(inputs)
    t0 = np.asarray(inputs["tier0_features"], dtype=np.float32)
    rb = np.asarray(inputs["robot_features"], dtype=np.float32)
    tk = np.asarray(inputs["track_features"], dtype=np.float32)

    in_maps = []
    for c in range(N_CORES):
        s = slice(c * B_C, (c + 1) * B_C)
        in_maps.append({
            "xt": _pack_x(tk[s].reshape(B_C * NT, DT), DT, QT),
            "xr": _pack_x(rb[s].reshape(B_C * NR, DR), DR, QR),
            "t0": np.ascontiguousarray(t0[s].T).astype(MMDT_NP),
            "wts": wts, "wbf": wbf, "bs": bs,
        })

    res = run_bass_kernel_spmd(nc, in_maps, core_ids=list(range(N_CORES)))
    out = np.concatenate([r["y"][0] for r in res.results])
    return out.astype(np.float32)


if __name__ == "__main__":
    rng = np.random.default_rng(0)
    fake = {
        "tier0_features": rng.standard_normal((B, 44), dtype=np.float32),
        "robot_features": rng.standard_normal((B, NR, DR), dtype=np.float32),
        "track_features": rng.standard_normal((B, NT, DT), dtype=np.float32),
    }
    for n, sh in (("rw1", (6, 64)), ("rw2", (64, 64)), ("rw3", (64, 32)),
                  ("tw1", (7, 64)), ("tw2", (64, 64)), ("tw3", (64, 32)),
                  ("mw1", (108, 128)), ("mw2", (128, 128)),
                  ("mw3", (128, 64)), ("mw4", (64, 1))):
        fake[n] = rng.standard_normal(sh, dtype=np.float32) * 0.2
    for n, sh in (("rb1", 64), ("rb2", 64), ("rb3", 32),
                  ("tb1", 64), ("tb2", 64), ("tb3", 32),
                  ("mb1", 128), ("mb2", 128), ("mb3", 64), ("mb4", 1)):
        fake[n] = rng.standard_normal((sh,), dtype=np.float32) * 0.1
    y = kernel(**fake)
    print("kernel out:", y.shape, y[:4])



# revision 39
# speedup vs baseline: 1.0025x; 1.0005x over previous
"""Trainium2 Bass kernel for nn_CentralizedCritic (pooling critic net).

Data-parallel over 8 NeuronCores: each core handles B_c=2048 batch rows.

Per-core math (matches the jax reference):
  robot_emb = setenc(robot[b], rw*)  -> [B,32]   (mean+max pool over 64)
  track_emb = setenc(track[b], tw*)  -> [B,32]   (mean+max pool over 128)
  c = [tier0, robot_emb, track_emb]  -> [B,108]
  y = mlp(c)                         -> [B]

On-chip mapping:
  - Activations kept transposed [feat, rows]; 2 batch-halves packed on the
    partition dim via block-diag weights (K=2*d_in, M=2*d_hidden=128).
  - x^T pre-packed on host into 4 row-groups at partition offsets {0,32,64,96}
    so L1 matmuls row-tile the PE array. bf16 throughout (rel err ~7e-3).
  - Both branches flattened into one desc list and software-pipelined 3 deep
    (PE per iter: L1(i), L2(i-1), L3(i-2)) so the PE never waits on a
    same-iteration evacuation.
  - relu+bias fused into the PSUM->SBUF evacuation; evacs column-split
    between ACT and DVE (SPLITS) so ACT-busy ~= DVE-busy (reduces are
    DVE-only: ACT and DVE are the only PSUM-capable engines, which sets
    the ~3.6us/tile steady-state floor).
  - mean-pool: DVE reduce_sum from PSUM; max-pool: DVE reduce_max from PSUM.
  - e-bias (rb3/tb3) folded into the head-L1 bias on host.
  - branch combine+scatter issued as soon as its pair-columns are final
    (track fully + robot cols [0,192) hidden under the steady state); head
    runs layer-major with ACT/DVE-alternating evacs so its 4 col-tiles
    pipeline.
"""

import sys

sys.path.insert(0, "/opt/trn_rl_repo")

import numpy as np
import ml_dtypes

import concourse.bass as bass  # noqa: F401  (bass must import before tile)
import concourse.mybir as mybir
import concourse.tile as tile
from concourse import bacc
from concourse.bass_utils import run_bass_kernel_spmd

F32R = mybir.dt.float32r
F32 = mybir.dt.float32
BF16 = mybir.dt.bfloat16
AF = mybir.ActivationFunctionType
ALU = mybir.AluOpType
AX = mybir.AxisListType

N_CORES = 8
B = 16384
B_C = B // N_CORES          # 2048 batch rows per core
HALF = B_C // 2             # 1024 (2-row packing pairs b and b+HALF)
NR, DR = 64, 6              # robot set size / feature dim
NT, DT = 128, 7             # track set size / feature dim
CT = B_C * NT // 2          # 131072 packed track cols per core
CR = B_C * NR // 2          # 65536 packed robot cols per core
QT = CT // 4                # 32768 cols per track row-group
QR = CR // 4                # 16384 cols per robot row-group
CHUNK = int(__import__("os").environ.get("CHUNK", "2048"))  # dma chunk cols
NTILE = 512                 # matmul free dim
# DVE column-split per evac slot [e1h0, e1h1, e2h0, e2h1]: how many of the
# NTILE columns of that evac go to DVE (rest on ACT). 0=all ACT, 512=all DVE.
SPLITS = tuple(int(x) for x in
               __import__("os").environ.get("SPLITS", "0,0,32,312").split(","))
LOOKAHEAD = int(__import__("os").environ.get("LOOKAHEAD", "5"))

# const-block column layout in "wts" [128, 840] (f32r)
W1T_C, W1R_C, W2T_C, W2R_C = 0, 128, 256, 384
MW1_C, MW2_C, MW3_C, MW4_C = 512, 640, 768, 832
WTS_W = 840
# "bs" [128, 8] (f32) bias columns
BS_TB1, BS_RB1, BS_TB2, BS_RB2, BS_MB1, BS_MB2, BS_MB3, BS_MB4 = range(8)

_CACHE = {}

import os
PROBE_NO_REDUCE = os.environ.get("PROBE_NO_REDUCE") == "1"
POOL_EVAC = os.environ.get("POOL_EVAC", "0") == "1"
PROBE_NO_EVAC = os.environ.get("PROBE_NO_EVAC") == "1"
PROBE_NO_MM3 = os.environ.get("PROBE_NO_MM3") == "1"
HBUF_BUFS = int(os.environ.get("HBUF_BUFS", "3"))
XC_BUFS = int(os.environ.get("XC_BUFS", "4"))
PS_BUFS = int(os.environ.get("PS_BUFS", "3"))
PS3_BUFS = int(os.environ.get("PS3_BUFS", "1"))  # 0 = share "ps" pool
DVE_EVAC_MOD = int(os.environ.get("DVE_EVAC_MOD", "3"))
MMDT_NAME = os.environ.get("MMDT", "bf16")
SPLIT_L3 = os.environ.get("SPLIT_L3", "0") == "1"
EVAC_ASSIGN = os.environ.get("EVAC_ASSIGN", "rr")  # rr | e1
MMDT = mybir.dt.bfloat16 if MMDT_NAME == "bf16" else mybir.dt.float32r
MMDT_NP = ml_dtypes.bfloat16 if MMDT_NAME == "bf16" else np.float32


def _build_bass():
    nc = bacc.Bacc("TRN2", target_bir_lowering=False, debug=False,
                   num_devices=N_CORES)
    xt_d = nc.dram_tensor("xt", [128, QT], MMDT, kind="ExternalInput")
    xr_d = nc.dram_tensor("xr", [128, QR], MMDT, kind="ExternalInput")
    t0_d = nc.dram_tensor("t0", [44, B_C], MMDT, kind="ExternalInput")
    wts_d = nc.dram_tensor("wts", [128, WTS_W], MMDT, kind="ExternalInput")
    wbf_d = nc.dram_tensor("wbf", [128, 128], BF16, kind="ExternalInput")
    bs_d = nc.dram_tensor("bs", [128, 8], F32, kind="ExternalInput")
    y_d = nc.dram_tensor("y", [1, B_C], F32, kind="ExternalOutput")

    with tile.TileContext(nc) as tc:
        with (
            tc.tile_pool(name="consts", bufs=1) as consts,
            tc.tile_pool(name="xchunks", bufs=XC_BUFS) as xchunks,
            tc.tile_pool(name="hbuf", bufs=HBUF_BUFS) as hbuf,
            tc.tile_pool(name="acc", bufs=1) as acc,
            tc.tile_pool(name="head", bufs=2) as head,
            tc.tile_pool(name="ps", bufs=PS_BUFS, space="PSUM") as ps,
            tc.tile_pool(name="ps3p", bufs=max(PS3_BUFS, 1),
                         space="PSUM") as ps3p,
        ):
            wts = consts.tile([128, WTS_W], MMDT)
            wbf = consts.tile([128, 128], BF16)
            bs = consts.tile([128, 8], F32)
            nc.sync.dma_start(out=wts[:, 0:128], in_=wts_d[:, 0:128])
            nc.sync.dma_start(out=wts[:, 128:256], in_=wts_d[:, 128:256])
            nc.sync.dma_start(out=bs[:], in_=bs_d[:])

            cT = acc.tile([108, B_C], MMDT)

            # pooled accumulators: col = 256*J + 32*ch + 4*t + om (track)
            #                      col = 256*J + 64*ch + 8*t + om (robot)
            esum_t = acc.tile([128, 512], F32, tag="esum_t")
            emax_t = acc.tile([128, 512], F32, tag="emax_t")
            esum_r = acc.tile([128, 512], F32, tag="esum_r")
            emax_r = acc.tile([128, 512], F32, tag="emax_r")

            # ---- unified descriptor list over both branches ----
            # each desc = one NTILE-wide tile step of a branch
            descs = []
            chunks = []  # (x_d, col0, first_desc_idx)
            for (x_d, qcols, k2, w1_c, w2_c, w3_c, bs1, bs2, nseg,
                 esum, emax) in (
                    (xt_d, QT, 2 * DT, W1T_C, W2T_C, 0, BS_TB1, BS_TB2, NT,
                     esum_t, emax_t),
                    (xr_d, QR, 2 * DR, W1R_C, W2R_C, 64, BS_RB1, BS_RB2, NR,
                     esum_r, emax_r)):
                nchunks = qcols // CHUNK
                tpc = CHUNK // NTILE
                tg = 0
                for ch in range(nchunks):
                    chunks.append((x_d, ch * CHUNK, len(descs)))
                    for t in range(tpc):
                        descs.append(dict(
                            chunk=len(chunks) - 1, cs=slice(t * NTILE,
                                                            (t + 1) * NTILE),
                            k2=k2, w1_c=w1_c, w2_c=w2_c, w3_c=w3_c,
                            bs1=bs1, bs2=bs2, nseg=nseg, esum=esum,
                            emax=emax, nb=NTILE // nseg,
                            base=(NTILE // nseg) * tg))
                        tg += 1

            chunk_tiles = {}
            issued = [0]  # chunks issued so far

            def issue_chunk(ci):
                x_d, c0, _ = chunks[ci]
                xc = xchunks.tile([128, CHUNK], MMDT, tag="xc")
                if ci == 0:
                    # split the first chunk so desc 0 unblocks after the
                    # first NTILE columns land (range-level dep tracking)
                    for q0 in range(0, CHUNK, NTILE):
                        nc.sync.dma_start(
                            out=xc[:, q0:q0 + NTILE],
                            in_=x_d[:, c0 + q0:c0 + q0 + NTILE])
                else:
                    nc.sync.dma_start(out=xc[:], in_=x_d[:, c0:c0 + CHUNK])
                chunk_tiles[ci] = xc

            def evac(pshalf, htile, half, bias_col, dve_cols):
                """relu+bias PSUM->SBUF; column-split ACT/DVE."""
                ac = NTILE - dve_cols
                if dve_cols > 0:
                    eng = nc.gpsimd if POOL_EVAC else nc.vector
                    eng.tensor_scalar(
                        out=htile[:, 2 * half:2 * half + 2, ac:NTILE],
                        in0=pshalf[:, :, ac:NTILE],
                        scalar1=bs[:, bias_col:bias_col + 1],
                        scalar2=0.0, op0=ALU.add, op1=ALU.max)
                if ac > 0:
                    nc.scalar.activation(
                        out=htile[:, 2 * half:2 * half + 2, 0:ac],
                        in_=pshalf[:, :, 0:ac], func=AF.Relu,
                        bias=bs[:, bias_col:bias_col + 1], scale=1.0)

            def stage1(d):
                """L1 matmuls + evac1 -> h1."""
                ci = d["chunk"]
                while issued[0] <= min(ci + 1, len(chunks) - 1):
                    issue_chunk(issued[0])
                    issued[0] += 1
                xc = chunk_tiles[ci]
                h1 = hbuf.tile([128, 4, NTILE], MMDT, tag="h1")
                d["h1"] = h1
                pstiles = []
                for half in range(2):
                    p = ps.tile([128, 2, NTILE], F32, tag="ps")
                    pstiles.append(p)
                    for j in range(2):
                        q = 2 * half + j
                        nc.tensor.matmul(
                            p[:, j, :],
                            wts[32 * q:32 * q + d["k2"],
                                d["w1_c"]:d["w1_c"] + 128],
                            xc[32 * q:32 * q + d["k2"], d["cs"]],
                            start=True, stop=True,
                            tile_position=(32 * q, 0))
                for half in range(2):
                    evac(pstiles[half], h1, half, d["bs1"], SPLITS[half])

            def stage2(d):
                """L2 matmuls + evac2 -> h2."""
                h1 = d.pop("h1")
                h2 = hbuf.tile([128, 4, NTILE], BF16, tag="h2")
                d["h2"] = h2
                pstiles = []
                for half in range(2):
                    p = ps.tile([128, 2, NTILE], F32, tag="ps")
                    pstiles.append(p)
                    for j in range(2):
                        q = 2 * half + j
                        nc.tensor.matmul(
                            p[:, j, :],
                            wts[:, d["w2_c"]:d["w2_c"] + 128],
                            h1[:, q, :], start=True, stop=True)
                for half in range(2):
                    evac(pstiles[half], h2, half, d["bs2"], SPLITS[2 + half])

            def stage3(d):
                """L3 (col-paired) + pooling reduces."""
                h2 = d.pop("h2")
                nb, nseg = d["nb"], d["nseg"]
                pool3 = ps3p if PS3_BUFS > 0 else ps
                ps3 = pool3.tile([128, 2 * nb * nseg], F32,
                                 tag="ps3" if PS3_BUFS else "ps")
                for q in range(4):
                    J, blk = q // 2, q % 2
                    nc.tensor.matmul(
                        ps3[64 * blk:64 * blk + 64,
                            J * NTILE:(J + 1) * NTILE],
                        wbf[:, d["w3_c"]:d["w3_c"] + 64],
                        h2[:, q, :], start=True, stop=True,
                        tile_position=(0, 64 * blk))
                p3r = ps3.rearrange("p (a b c) -> p (a b) c", a=2, b=nb)
                base = d["base"]
                sview = d["esum"].rearrange("p (J r) -> p J r", J=2)[
                    :, :, base:base + nb]
                mview = d["emax"].rearrange("p (J r) -> p J r", J=2)[
                    :, :, base:base + nb]
                if not PROBE_NO_REDUCE:
                    nc.vector.reduce_sum(out=sview, in_=p3r[:], axis=AX.X)
                    nc.vector.reduce_max(out=mview, in_=p3r[:], axis=AX.X)

            emb_t = acc.tile([128, 512], MMDT, tag="emb_t")
            emb_r = acc.tile([128, 512], MMDT, tag="emb_r")

            def combine_and_scatter(esum, emax, emb, nseg, row0, p0, p1,
                                    parallel_q=False):
                """emb = esum/(2*nseg) + 0.5*emax for pair-cols [p0, p1) of
                each J half, then scatter into cT. Split so most of it can
                issue before a branch's last reduce."""
                npair = p1 - p0
                eview = lambda t: t.rearrange("p (J x) -> p J x", J=2)[
                    :, :, p0:p1]
                tmp = hbuf.tile([128, 512], F32, tag="tmp")
                tv = tmp.rearrange("p (J x) -> p J x", J=2)[:, :, p0:p1]
                nc.vector.tensor_scalar(out=tv, in0=eview(esum),
                                        scalar1=1.0 / (2.0 * nseg),
                                        scalar2=None, op0=ALU.mult)
                nc.vector.scalar_tensor_tensor(
                    out=eview(emb), in0=eview(emax), scalar=0.5, in1=tv,
                    op0=ALU.mult, op1=ALU.add)
                # scatter, J-dim merged: 4 DMAs per call
                embJ = emb.rearrange("p (J x) -> p J x", J=2)[:, :, p0:p1]
                cTv = cT[row0:row0 + 32, :].rearrange(
                    "p (h J blk x) -> p h J blk x", h=2, J=2, blk=2)
                engs = ([nc.sync, nc.scalar, nc.gpsimd, nc.sync]
                        if parallel_q else [nc.sync] * 4)
                for blk in range(2):
                    for h in range(2):
                        engs[2 * blk + h].dma_start(
                            out=cTv[:, h, :, blk, p0:p1],
                            in_=embJ[64 * blk + 32 * h:
                                     64 * blk + 32 * h + 32, :, :])

            n = len(descs)
            ntrack = QT // NTILE  # first ntrack descs are the track branch
            # prime the pipeline: x chunks first, then non-critical consts
            issue_chunk(0)
            issue_chunk(1)
            issued[0] = 2
            nc.sync.dma_start(out=wts[:, 256:WTS_W], in_=wts_d[:, 256:WTS_W])
            nc.sync.dma_start(out=wbf[:], in_=wbf_d[:])
            nc.sync.dma_start(out=cT[0:44, :], in_=t0_d[:])
            for i in range(n + 2):
                if i < n:
                    stage1(descs[i])
                if 1 <= i <= n:
                    stage2(descs[i - 1])
                if 2 <= i <= n + 1:
                    stage3(descs[i - 2])
                if i - 2 == ntrack - 1:
                    combine_and_scatter(esum_t, emax_t, emb_t, NT, 76,
                                        0, 256)
                if i - 2 == ntrack + 23:
                    # robot pair-cols [0,192) are final after robot desc 23
                    combine_and_scatter(esum_r, emax_r, emb_r, NR, 44,
                                        0, 192)
                if i - 2 == ntrack + 30:
                    combine_and_scatter(esum_r, emax_r, emb_r, NR, 44,
                                        192, 248)
            combine_and_scatter(esum_r, emax_r, emb_r, NR, 44, 248, 256,
                                parallel_q=True)

            # keep the PE p-state warm across the drain->head gap with
            # dummy matmuls on resident constants (results never read)
            NWARM = int(os.environ.get("NWARM", "0"))
            if NWARM:
                warm = ps.tile([128, 2, NTILE], F32, tag="ps")
                for w in range(NWARM):
                    nc.tensor.matmul(warm[:, w % 2, :], wts[:, 0:128],
                                     wts[:, 256:768], start=True, stop=True)

            # ---- head MLP 108 -> 128 -> 128 -> 64 -> 1, layer-major ----
            # evacs alternate ACT/DVE so the 4 col-tiles pipeline.
            y_sb = acc.tile([1, B_C], F32, tag="y")
            NHT = B_C // NTILE  # 4 col-tiles

            def head_evac(pst, dsts, s, bias_col, prange=128, func=AF.Relu):
                if s % 2 == 0:
                    nc.scalar.activation(
                        out=dsts, in_=pst[:], func=func,
                        bias=bs[0:prange, bias_col:bias_col + 1], scale=1.0)
                elif func == AF.Relu:
                    nc.vector.tensor_scalar(
                        out=dsts, in0=pst[:],
                        scalar1=bs[0:prange, bias_col:bias_col + 1],
                        scalar2=0.0, op0=ALU.add, op1=ALU.max)
                else:
                    nc.vector.tensor_scalar(
                        out=dsts, in0=pst[:],
                        scalar1=bs[0:prange, bias_col:bias_col + 1],
                        scalar2=None, op0=ALU.add)

            hh1 = head.tile([128, B_C], MMDT, tag="hh1")
            hh2 = head.tile([128, B_C], MMDT, tag="hh2")
            hh3 = head.tile([64, B_C], MMDT, tag="hh3")
            layers = (
                (lambda s: wts[0:108, MW1_C:MW1_C + 128],
                 lambda s: cT[:, s * NTILE:(s + 1) * NTILE],
                 hh1, BS_MB1, 128),
                (lambda s: wts[:, MW2_C:MW2_C + 128],
                 lambda s: hh1[:, s * NTILE:(s + 1) * NTILE],
                 hh2, BS_MB2, 128),
                (lambda s: wts[:, MW3_C:MW3_C + 64],
                 lambda s: hh2[:, s * NTILE:(s + 1) * NTILE],
                 hh3, BS_MB3, 64),
            )
            for wfn, infn, dst, bcol, prange in layers:
                psl = []
                for s in range(NHT):
                    p = ps.tile([prange, NTILE], F32, tag="ps")
                    psl.append(p)
                    nc.tensor.matmul(p[:], wfn(s), infn(s),
                                     start=True, stop=True)
                for s in range(NHT):
                    head_evac(psl[s],
                              dst[0:prange, s * NTILE:(s + 1) * NTILE],
                              s, bcol, prange)
            psl = []
            for s in range(NHT):
                p = ps.tile([1, NTILE], F32, tag="ps")
                psl.append(p)
                nc.tensor.matmul(p[:], wts[0:64, MW4_C:MW4_C + 1],
                                 hh3[:, s * NTILE:(s + 1) * NTILE],
                                 start=True, stop=True)
            for s in range(NHT):
                head_evac(psl[s], y_sb[:, s * NTILE:(s + 1) * NTILE],
                          s, BS_MB4, 1, AF.Identity)
            nc.sync.dma_start(out=y_d[:, 0:B_C // 2],
                              in_=y_sb[:, 0:B_C // 2])
            nc.sync.dma_start(out=y_d[:, B_C // 2:B_C],
                              in_=y_sb[:, B_C // 2:B_C])

    nc.compile()
    return nc


def _pack_x(x, d, qcols):
    """x [rows, d] (rows = B_c*nseg, b-major) -> [128, qcols] with 4
    row-groups at partition offsets {0,32,64,96}; 2-row packing pairs
    row r with row r + rows/2."""
    rows = x.shape[0]
    half = rows // 2
    packed = np.concatenate([x[:half].T, x[half:].T], axis=0)  # [2d, half]
    out = np.zeros((128, qcols), dtype=MMDT_NP)
    for q in range(4):
        out[32 * q:32 * q + 2 * d] = packed[:, q * qcols:(q + 1) * qcols]
    return np.ascontiguousarray(out)


def _blockdiag2(w):
    """w [d, m] -> [2d, 2m] block-diagonal."""
    d, m = w.shape
    out = np.zeros((2 * d, 2 * m), dtype=np.float32)
    out[:d, :m] = w
    out[d:, m:] = w
    return out


def _build_consts(i):
    np32 = lambda a: np.asarray(a, dtype=np.float32)
    wts = np.zeros((128, WTS_W), dtype=np.float32)
    # L1 lhsT blocks replicated at the 4 row-group offsets
    bd1t = _blockdiag2(np32(i["tw1"]))   # [14, 128]
    bd1r = _blockdiag2(np32(i["rw1"]))   # [12, 128]
    for q in range(4):
        wts[32 * q:32 * q + 14, W1T_C:W1T_C + 128] = bd1t
        wts[32 * q:32 * q + 12, W1R_C:W1R_C + 128] = bd1r
    wts[:, W2T_C:W2T_C + 128] = _blockdiag2(np32(i["tw2"]))
    wts[:, W2R_C:W2R_C + 128] = _blockdiag2(np32(i["rw2"]))
    wts[0:108, MW1_C:MW1_C + 128] = np32(i["mw1"])
    wts[:, MW2_C:MW2_C + 128] = np32(i["mw2"])
    wts[:, MW3_C:MW3_C + 64] = np32(i["mw3"])
    wts[0:64, MW4_C:MW4_C + 1] = np32(i["mw4"])
    wts = wts.astype(MMDT_NP)

    wbf = np.zeros((128, 128), dtype=np.float32)
    wbf[:, 0:64] = _blockdiag2(np32(i["tw3"]))
    wbf[:, 64:128] = _blockdiag2(np32(i["rw3"]))
    wbf = wbf.astype(ml_dtypes.bfloat16)

    bs = np.zeros((128, 8), dtype=np.float32)
    bs[:, BS_TB1] = np.concatenate([np32(i["tb1"]), np32(i["tb1"])])
    bs[:, BS_RB1] = np.concatenate([np32(i["rb1"]), np32(i["rb1"])])
    bs[:, BS_TB2] = np.concatenate([np32(i["tb2"]), np32(i["tb2"])])
    bs[:, BS_RB2] = np.concatenate([np32(i["rb2"]), np32(i["rb2"])])
    # fold pooled e-bias into head L1 bias: c@mw1 picks up b3@mw1 rows
    mb1p = (np32(i["mb1"])
            + np32(i["rb3"]) @ np32(i["mw1"])[44:76]
            + np32(i["tb3"]) @ np32(i["mw1"])[76:108])
    bs[:, BS_MB1] = mb1p
    bs[:, BS_MB2] = np32(i["mb2"])
    bs[0:64, BS_MB3] = np32(i["mb3"])
    bs[0:1, BS_MB4] = np32(i["mb4"])
    return wts, wbf, bs


def kernel(**inputs) -> np.ndarray:
    if "nc" not in _CACHE:
        _CACHE["nc"] = _build_bass()
    nc = _CACHE["nc"]

    wts, wbf, bs = _build_consts(inputs)
    t0 = np.asarray(inputs["tier0_features"], dtype=np.float32)
    rb = np.asarray(inputs["robot_features"], dtype=np.float32)
    tk = np.asarray(inputs["track_features"], dtype=np.float32)

    in_maps = []
    for c in range(N_CORES):
        s = slice(c * B_C, (c + 1) * B_C)
        in_maps.append({
            "xt": _pack_x(tk[s].reshape(B_C * NT, DT), DT, QT),
            "xr": _pack_x(rb[s].reshape(B_C * NR, DR), DR, QR),
            "t0": np.ascontiguousarray(t0[s].T).astype(MMDT_NP),
            "wts": wts, "wbf": wbf, "bs": bs,
        })

    res = run_bass_kernel_spmd(nc, in_maps, core_ids=list(range(N_CORES)))
    out = np.concatenate([r["y"][0] for r in res.results])
    return out.astype(np.float32)


if __name__ == "__main__":
    rng = np.random.default_rng(0)
    fake = {
        "tier0_features": rng.standard_normal((B, 44), dtype=np.float32),
        "robot_features": rng.standard_normal((B, NR, DR), dtype=np.float32),
        "track_features": rng.standard_normal((B, NT, DT), dtype=np.float32),
    }
    for n, sh in (("rw1", (6, 64)), ("rw2", (64, 64)), ("rw3", (64, 32)),
                  ("tw1", (7, 64)), ("tw2", (64, 64)), ("tw3", (64, 32)),
                  ("mw1", (108, 128)), ("mw2", (128, 128)),
                  ("mw3", (128, 64)), ("mw4", (64, 1))):
        fake[n] = rng.standard_normal(sh, dtype=np.float32) * 0.2
    for n, sh in (("rb1", 64), ("rb2", 64), ("rb3", 32),
                  ("tb1", 64), ("tb2", 64), ("tb3", 32),
                  ("mb1", 128), ("mb2", 128), ("mb3", 64), ("mb4", 1)):
        fake[n] = rng.standard_normal((sh,), dtype=np.float32) * 0.1
    y = kernel(**fake)
    print("kernel out:", y.shape, y[:4])

